# revision 9
# baseline (speedup 1.0000x reference)
"""Trainium2 Bass kernel: batched GRU encoder + beam-2 search decoder.

8 NeuronCores SPMD: encoder replicated on all cores; decoder output projection
sharded 8-way over the vocab (4000 rows/core, padded to 4096). Each decode
step every core computes fp32 logits for its shard, screens local top-2
(+forced vocab-0/1) candidates via the DVE max8 unit, AllGathers a small
payload, and all cores run an identical combine (global top-2 with
composite-index tie-breaking that matches jax.lax.top_k, including the
fp32-underflow regime where beam probabilities collapse to 0).
"""
import sys, types
try:
    from trn_agent_boot.trn_boot import _ntff_profile_via_ctypes
    _hook = _ntff_profile_via_ctypes('/opt/axon/libaxon_pjrt.so')
    _m = types.ModuleType("antenv.axon_hooks")
    _m.get_axon_ntff_profile_hook = lambda: _hook
    sys.modules.setdefault("antenv.axon_hooks", _m)
except Exception:
    pass

import numpy as np
import concourse.bass as bass
import concourse.mybir as mybir
import concourse.tile as tile
from concourse import bacc
from concourse.bass_types import AP
from concourse.bass_utils import run_bass_kernel_spmd

dt = mybir.dt
F32, I32, U32 = dt.float32, dt.int32, dt.uint32
AF = mybir.ActivationFunctionType
OP = mybir.AluOpType

NC = 8
V, E, H, L, B = 32000, 256, 512, 128, 16
G = 3 * H
T = 32
SOS, EOS = 2, 3
VS = V // NC
VSP = 4096
NEGC = -2.0e30
BIGC = 9.0e9


def apb(t_ap, dims):
    return AP(t_ap.tensor, t_ap.offset, [list(d) for d in dims])


def build_kernel():
    nc = bacc.Bacc("TRN2", target_bir_lowering=False, debug=False,
                   enable_asserts=False, num_devices=NC)
    ins = {}

    def inp(name, shape, d=F32):
        t = nc.dram_tensor(name, list(shape), d, kind="ExternalInput")
        ins[name] = t
        return t

    inp("srcw", [128, 16], I32)
    inp("sosidx", [32, 1], I32)
    inp("embenc", [V, E])
    inp("embdec", [V, E])
    inp("wihte", [E, G])
    inp("whhte", [H, G])
    inp("krz_enc", [16, 1024])
    inp("kin_enc", [16, 512])
    inp("khn_enc", [16, 512])
    inp("wihtde", [E, G])
    inp("wihtdc", [H, G])
    inp("whhtd", [H, G])
    inp("krz_dec", [32, 1024])
    inp("kin_dec", [32, 512])
    inp("khn_dec", [32, 512])
    inp("wote", [768, VSP])
    inp("wotc", [H, VSP])
    inp("bout", [1, VSP])
    inp("ident", [128, 128])
    inp("sk0", [16, 32])       # sk0[b, j] = (j == 2b)
    inp("sk1", [16, 32])
    inp("skT0", [32, 16])      # transposed versions
    inp("skT1", [32, 16])
    inp("rowbase", [32, 1])
    inp("rowid", [32, 1])
    inp("kofs", [32, 1])
    inp("sadd", [16, 64])
    inp("bigt", [16, 64])
    inp("negt", [16, 64])

    out_tok = nc.dram_tensor("out_tok", [16, T], I32, kind="ExternalOutput")
    out_sc = nc.dram_tensor("out_sc", [16, T], F32, kind="ExternalOutput")

    with tile.TileContext(nc) as tc:
        _body(nc, tc, ins, out_tok, out_sc)
    nc.compile()
    return nc


def _chunks(n):
    return n // 128


def _body(nc, tc, ins, out_tok, out_sc):
    import contextlib
    ctx = contextlib.ExitStack()
    wp = ctx.enter_context(tc.tile_pool(name="wts", bufs=1))
    wk = ctx.enter_context(tc.tile_pool(name="work", bufs=1))
    ps = ctx.enter_context(tc.tile_pool(name="ps", bufs=2, space="PSUM"))
    psl = ctx.enter_context(tc.tile_pool(name="psl", bufs=1, space="PSUM"))
    dr = ctx.enter_context(tc.tile_pool(name="dram", bufs=2, space="DRAM"))

    def loadc(name, rows, cols, pool=wp, d=F32):
        """load a [rows, cols] DRAM tensor as row-chunk tiles of [128, cols]"""
        tiles = []
        for c in range(_chunks(rows)):
            s = pool.tile([128, cols], d, name=f"ld_{name}_{c}")
            nc.sync.dma_start(s[:], ins[name].ap()[c * 128:(c + 1) * 128, :])
            tiles.append(s)
        return tiles

    def load1(name, shape, pool=wp, d=F32):
        s = pool.tile(list(shape), d, name="ld_" + name)
        nc.sync.dma_start(s[:], ins[name].ap())
        return s

    khn_d = load1("khn_dec", [32, 512])
    ident = load1("ident", [128, 128])
    sk0 = load1("sk0", [16, 32])
    sk1 = load1("sk1", [16, 32])
    skT0 = load1("skT0", [32, 16])
    skT1 = load1("skT1", [32, 16])
    rowbase = load1("rowbase", [32, 1])
    rowid = load1("rowid", [32, 1])
    kofs = load1("kofs", [32, 1])
    sadd = load1("sadd", [16, 64])
    bigt = load1("bigt", [16, 64])
    negt = load1("negt", [16, 64])
    sosidx = load1("sosidx", [32, 1], d=I32)
    onesr = wp.tile([1, 32], F32, name="onesr")
    nc.vector.memset(onesr[:], 1.0)

    ht = wp.tile([128, 4, 16], F32, name="ht")
    nc.vector.memset(ht[:], 0.0)

    # ================= encoder (scoped pools freed afterwards) =================
    with tc.tile_pool(name="encw", bufs=1) as ep:
        wihte = loadc("wihte", E, G, pool=ep)
        whhte = loadc("whhte", H, G, pool=ep)
        krz_e = load1("krz_enc", [16, 1024], pool=ep)
        kin_e = load1("kin_enc", [16, 512], pool=ep)
        khn_e = load1("khn_enc", [16, 512], pool=ep)
        srcw = load1("srcw", [128, 16], pool=ep, d=I32)
        xt = [ep.tile([128, 2048], F32, name=f"xt{c}") for c in range(2)]
        for j in range(16):
            xr = wk.tile([128, E], F32, tag="xrow")
            nc.gpsimd.indirect_dma_start(
                out=xr[:], out_offset=None, in_=ins["embenc"].ap(),
                in_offset=bass.IndirectOffsetOnAxis(ap=srcw[:, j:j + 1], axis=0))
            for c in range(2):
                pt = ps.tile([128, 512], F32, tag="pscr")
                nc.tensor.transpose(pt[:, 0:128], xr[:, c * 128:(c + 1) * 128], ident[:])
                nc.vector.tensor_copy(xt[c][:, j * 128:(j + 1) * 128], pt[:, 0:128])

        h_r = ep.tile([16, H], F32, name="h_r")
        nc.vector.memset(h_r[:], 0.0)

        for t in range(L):
            rz = psl.tile([16, 1024], F32, tag="psA", name=f"erz{t % 2}")
            bn = psl.tile([16, 512], F32, tag="psB", name=f"ebn{t % 2}")
            an = psl.tile([16, 512], F32, tag="psC", name=f"ean{t % 2}")
            for c in range(4):
                hc = ht[:, c, :]
                wc = whhte[c]
                for n in range(2):
                    nc.tensor.matmul(rz[:, n * 512:(n + 1) * 512], hc,
                                     wc[:, n * 512:(n + 1) * 512],
                                     start=(c == 0), stop=False)
                nc.tensor.matmul(bn[:], hc, wc[:, 1024:1536],
                                 start=(c == 0), stop=(c == 3))
            for c in range(2):
                xc = xt[c][:, t * 16:(t + 1) * 16]
                wc = wihte[c]
                for n in range(2):
                    nc.tensor.matmul(rz[:, n * 512:(n + 1) * 512], xc,
                                     wc[:, n * 512:(n + 1) * 512],
                                     start=False, stop=(c == 1))
                nc.tensor.matmul(an[:], xc, wc[:, 1024:1536],
                                 start=(c == 0), stop=(c == 1))

            rzs = wk.tile([16, 1024], F32, tag="rzs")
            nc.vector.tensor_add(rzs[:], rz[:], krz_e[:])
            nc.scalar.activation(rzs[:], rzs[:], AF.Tanh, scale=0.5)
            nc.vector.tensor_scalar(rzs[:], rzs[:], 0.5, scalar2=0.5,
                                    op0=OP.mult, op1=OP.add)
            nt = wk.tile([16, 512], F32, tag="ntl")
            nc.vector.tensor_add(nt[:], bn[:], khn_e[:])
            nc.vector.tensor_mul(nt[:], nt[:], rzs[:, 0:512])
            nc.vector.tensor_add(nt[:], nt[:], an[:])
            nc.vector.tensor_add(nt[:], nt[:], kin_e[:])
            nc.scalar.activation(nt[:], nt[:], AF.Tanh)
            hmn = wk.tile([16, 512], F32, tag="hmn")
            nc.vector.tensor_sub(hmn[:], h_r[:], nt[:])
            nc.vector.tensor_mul(hmn[:], hmn[:], rzs[:, 512:1024])
            nc.vector.tensor_add(h_r[:], nt[:], hmn[:])
            for c in range(4):
                pt = ps.tile([128, 512], F32, tag="pscr")
                nc.tensor.transpose(pt[:, 0:16], h_r[:, c * 128:(c + 1) * 128],
                                    ident[0:16, 0:16])
                nc.vector.tensor_copy(ht[:, c, :], pt[:, 0:16])

    # ================= decoder precompute =================
    wihtde = loadc("wihtde", E, G)
    whhtd = loadc("whhtd", H, G)
    # ctxT [128, 4, 32] with col = 2b + k  (duplicate h cols)
    ctT = wp.tile([128, 4, 32], F32, name="ctT")
    for c in range(4):
        base = ctT[:, c, 0:1]
        d_even = apb(base, [base.ap[0], [2, 16]])
        base1 = ctT[:, c, 1:2]
        d_odd = apb(base1, [base1.ap[0], [2, 16]])
        nc.vector.tensor_copy(d_even, ht[:, c, :])
        nc.vector.tensor_copy(d_odd, ht[:, c, :])
    ctx32 = wp.tile([32, H], F32, name="ctx32")
    for c in range(4):
        pt = ps.tile([128, 512], F32, tag="pscr")
        nc.tensor.transpose(pt[0:32, 0:128], ctT[:, c, :], ident[:])
        nc.vector.tensor_copy(ctx32[:, c * 128:(c + 1) * 128], pt[0:32, 0:128])

    # kdec = ctx @ WihT_ctx + biases
    kdec_rz = wp.tile([32, 1024], F32, name="kdecrz")
    kdec_n = wp.tile([32, 512], F32, name="kdecn")
    with tc.tile_pool(name="wc_tmp", bufs=2) as wct:
        krz_d = load1("krz_dec", [32, 1024], pool=wct)
        kin_d = load1("kin_dec", [32, 512], pool=wct)
        kd = psl.tile([32, 1024], F32, tag="psA", name="kdrz")
        kdn = psl.tile([32, 512], F32, tag="psB", name="kdn")
        for c in range(4):
            wcc = wct.tile([128, G], F32, tag="wcc")
            nc.sync.dma_start(wcc[:], ins["wihtdc"].ap()[c * 128:(c + 1) * 128, :])
            cc = ctT[:, c, :]
            for n in range(2):
                nc.tensor.matmul(kd[:, n * 512:(n + 1) * 512], cc,
                                 wcc[:, n * 512:(n + 1) * 512],
                                 start=(c == 0), stop=(c == 3))
            nc.tensor.matmul(kdn[:], cc, wcc[:, 1024:1536],
                             start=(c == 0), stop=(c == 3))
        nc.vector.tensor_add(kdec_rz[:], kd[:], krz_d[:])
        nc.vector.tensor_add(kdec_n[:], kdn[:], kin_d[:])

    # E = exp(ctx @ Wc.T + bout)
    et = wp.tile([32, VSP], F32, name="et")
    with tc.tile_pool(name="wotc_tmp", bufs=2) as wot:
        bout = load1("bout", [1, VSP], pool=wot)
        for n in range(8):
            pe = ps.tile([128, 512], F32, tag="pscr")
            for c in range(4):
                wcs = wot.tile([128, 512], F32, tag="wcs")
                nc.sync.dma_start(wcs[:], ins["wotc"].ap()[c * 128:(c + 1) * 128,
                                                           n * 512:(n + 1) * 512])
                nc.tensor.matmul(pe[0:32, :], ctT[:, c, :], wcs[:],
                                 start=(c == 0), stop=False)
            nc.tensor.matmul(pe[0:32, :], onesr[:], bout[:, n * 512:(n + 1) * 512],
                             start=False, stop=True)
            nc.scalar.activation(et[:, n * 512:(n + 1) * 512], pe[0:32, :], AF.Exp)

    big = ctx.enter_context(tc.tile_pool(name="big", bufs=1))
    wote = loadc("wote", 768, VSP, pool=big)

    # persistent decode state
    th = wp.tile([16, 2, T], F32, name="th")
    sh = wp.tile([16, 2, T], F32, name="sh")
    nc.vector.memset(th[:], float(EOS))
    nc.vector.memset(th[:, :, 0:1], float(SOS))
    nc.vector.memset(sh[:], 0.0)
    nc.vector.memset(sh[:, 0, 0:1], 1.0)
    prob = wp.tile([16, 2], F32, name="prob")
    nc.vector.memset(prob[:, 0:1], 1.0)
    nc.vector.memset(prob[:, 1:2], -1.0e30)

    hid = wp.tile([32, H], F32, name="hid")
    nc.vector.tensor_copy(hid[:], ctx32[:])
    ghb = wp.tile([32, G], F32, name="ghb")
    pex = big.tile([32, VSP], F32, name="pex")

    def gh_from(srcT):
        gh_ps = psl.tile([32, 1024], F32, tag="psA", name="ghps")
        gh_psn = psl.tile([32, 512], F32, tag="psB", name="ghpsn")
        for c in range(4):
            hc = srcT[:, c, :]
            wc = whhtd[c]
            for n in range(2):
                nc.tensor.matmul(gh_ps[:, n * 512:(n + 1) * 512], hc,
                                 wc[:, n * 512:(n + 1) * 512],
                                 start=(c == 0), stop=(c == 3))
            nc.tensor.matmul(gh_psn[:], hc, wc[:, 1024:1536],
                             start=(c == 0), stop=(c == 3))
        nc.vector.tensor_copy(ghb[:, 0:1024], gh_ps[:])
        nc.vector.tensor_copy(ghb[:, 1024:1536], gh_psn[:])

    gh_from(ctT)

    tok_i = wp.tile([32, 1], I32, name="tok_i")
    nc.vector.tensor_copy(tok_i[:], sosidx[:])
    newhT = wp.tile([128, 4, 32], F32, name="newhT")

    for t in range(1, T):
        er = wk.tile([32, E], F32, tag="erow")
        nc.gpsimd.indirect_dma_start(
            out=er[:], out_offset=None, in_=ins["embdec"].ap(),
            in_offset=bass.IndirectOffsetOnAxis(ap=tok_i[:, 0:1], axis=0))
        embT = wk.tile([128, 2, 32], F32, tag="embT")
        for c in range(2):
            pt = ps.tile([128, 512], F32, tag="pscr")
            nc.tensor.transpose(pt[0:32, 0:32] if False else pt[:, 0:32],
                                er[:, c * 128:(c + 1) * 128], ident[0:32, 0:32])
            nc.vector.tensor_copy(embT[:, c, :], pt[:, 0:32])

        arz = psl.tile([32, 1024], F32, tag="psA", name=f"arz{t % 2}")
        aan = psl.tile([32, 512], F32, tag="psB", name=f"aan{t % 2}")
        for c in range(2):
            xc = embT[:, c, :]
            wc = wihtde[c]
            for n in range(2):
                nc.tensor.matmul(arz[:, n * 512:(n + 1) * 512], xc,
                                 wc[:, n * 512:(n + 1) * 512],
                                 start=(c == 0), stop=(c == 1))
            nc.tensor.matmul(aan[:], xc, wc[:, 1024:1536],
                             start=(c == 0), stop=(c == 1))
        rzs = wk.tile([32, 1024], F32, tag="rzs")
        nc.vector.tensor_add(rzs[:], arz[:], kdec_rz[:])
        nc.vector.tensor_add(rzs[:], rzs[:], ghb[:, 0:1024])
        nc.scalar.activation(rzs[:], rzs[:], AF.Tanh, scale=0.5)
        nc.vector.tensor_scalar(rzs[:], rzs[:], 0.5, scalar2=0.5,
                                op0=OP.mult, op1=OP.add)
        ntl = wk.tile([32, 512], F32, tag="ntl")
        nc.vector.tensor_add(ntl[:], ghb[:, 1024:1536], khn_d[:])
        nc.vector.tensor_mul(ntl[:], ntl[:], rzs[:, 0:512])
        nc.vector.tensor_add(ntl[:], ntl[:], aan[:])
        nc.vector.tensor_add(ntl[:], ntl[:], kdec_n[:])
        nc.scalar.activation(ntl[:], ntl[:], AF.Tanh)
        newh = wk.tile([32, H], F32, tag="newh")
        nc.vector.tensor_sub(newh[:], hid[:], ntl[:])
        nc.vector.tensor_mul(newh[:], newh[:], rzs[:, 512:1024])
        nc.vector.tensor_add(newh[:], newh[:], ntl[:])
        for c in range(4):
            pt = ps.tile([128, 512], F32, tag="pscr")
            nc.tensor.transpose(pt[:, 0:32], newh[:, c * 128:(c + 1) * 128],
                                ident[0:32, 0:32])
            nc.vector.tensor_copy(newhT[:, c, :], pt[:, 0:32])

        gh_from(newhT)

        zq = wk.tile([32, 8], F32, tag="zq")
        for n in range(8):
            pl = ps.tile([128, 512], F32, tag="pscr")
            for c in range(2):
                nc.tensor.matmul(pl[0:32, :], embT[:, c, :],
                                 wote[c][:, n * 512:(n + 1) * 512],
                                 start=(c == 0), stop=False)
            for c in range(4):
                nc.tensor.matmul(pl[0:32, :], newhT[:, c, :],
                                 wote[2 + c][:, n * 512:(n + 1) * 512],
                                 start=False, stop=(c == 3))
            nc.scalar.activation(pex[:, n * 512:(n + 1) * 512], pl[0:32, :], AF.Exp)
        nc.vector.tensor_mul(pex[:], pex[:], et[:])
        for n in range(8):
            nc.vector.tensor_reduce(out=zq[:, n:n + 1],
                                    in_=pex[:, n * 512:(n + 1) * 512],
                                    op=OP.add, axis=mybir.AxisListType.X)
        zloc = wk.tile([32, 1], F32, tag="zloc")
        nc.vector.tensor_reduce(out=zloc[:], in_=zq[:], op=OP.add,
                                axis=mybir.AxisListType.X)

        mx = wk.tile([32, 8], F32, tag="mx")
        mi = wk.tile([32, 8], U32, tag="mi")
        nc.vector.max(out=mx[:], in_=pex[:])
        nc.vector.max_index(mi[:], mx[:], pex[:])

        M = wk.tile([32, 12], F32, tag="M")
        nc.vector.tensor_copy(M[:, 0:2], mx[:, 0:2])
        nc.vector.tensor_copy(M[:, 2:4], pex[:, 0:2])
        mif = wk.tile([32, 2], F32, tag="mif")
        nc.vector.tensor_copy(mif[:], mi[:, 0:2])
        nc.vector.tensor_add(M[:, 4:6], mif[:], kofs[:].to_broadcast([32, 2]))
        nc.vector.memset(M[:, 6:7], 0.0)
        nc.vector.memset(M[:, 7:8], 1.0)
        nc.vector.tensor_add(M[:, 6:8], M[:, 6:8], kofs[:].to_broadcast([32, 2]))
        nc.vector.tensor_copy(M[:, 8:9], zloc[:])
        nc.vector.memset(M[:, 9:12], 0.0)

        bi = dr.tile([32, 12], F32, tag="bi")
        bo = dr.tile([NC, 32, 12], F32, tag="bo")
        nc.sync.dma_start(bi[:], M[:])
        nc.gpsimd.collective_compute(
            "AllGather", OP.bypass, replica_groups=[list(range(NC))],
            ins=[bi.opt()], outs=[bo.opt()])
        Mm = wk.tile([32, NC, 12], F32, tag="Mm")
        nc.sync.dma_start(Mm[:], bo[:].rearrange("c r s -> r c s"))

        cps = ps.tile([128, 512], F32, tag="pscr")
        Mmf = Mm[:].rearrange("r c s -> r (c s)")
        nc.tensor.matmul(cps[0:16, 0:96], skT0[:], Mmf, start=True, stop=False)
        nc.tensor.matmul(cps[0:16, 96:192], skT1[:], Mmf, start=False, stop=True)
        Cf = wk.tile([16, 2, NC, 12], F32, tag="Cf")
        nc.vector.tensor_copy(Cf[:], cps[0:16, 0:192])

        Zk = wk.tile([16, 2], F32, tag="Zk")
        nc.vector.tensor_reduce(out=Zk[:], in_=Cf[:, :, :, 8], op=OP.add,
                                axis=mybir.AxisListType.X)
        rZ = wk.tile([16, 2], F32, tag="rZ")
        nc.vector.reciprocal(rZ[:], Zk[:])

        CV = wk.tile([16, 2, NC, 4], F32, tag="CV")
        rzb = apb(rZ[:], [rZ[:].ap[0], rZ[:].ap[1], [0, NC], [0, 4]])
        nc.vector.tensor_tensor(out=CV[:], in0=Cf[:, :, :, 0:4], in1=rzb, op=OP.mult)
        prb = apb(prob[:], [prob[:].ap[0], prob[:].ap[1], [0, NC], [0, 4]])
        nc.vector.tensor_tensor(out=CV[:], in0=CV[:], in1=prb, op=OP.mult)
        CVf = CV[:].rearrange("p a b c -> p (a b c)")
        nc.vector.tensor_add(CVf, CVf, sadd[:])
        CI4 = wk.tile([16, 2, NC, 4], F32, tag="CI")
        nc.vector.tensor_copy(CI4[:], Cf[:, :, :, 4:8])
        CI = CI4[:].rearrange("p a b c -> p (a b c)")

        m8 = wk.tile([16, 8], F32, tag="m8")
        nc.vector.max(out=m8[:], in_=CVf)
        eq = wk.tile([16, 64], U32, tag="eq")
        nc.vector.tensor_tensor(out=eq[:], in0=CVf,
                                in1=m8[:, 0:1].to_broadcast([16, 64]), op=OP.is_equal)
        isel = wk.tile([16, 64], F32, tag="isel")
        nc.vector.select(isel[:], eq[:], CI, bigt[:])
        c1 = wk.tile([16, 1], F32, tag="c1")
        nc.vector.tensor_reduce(out=c1[:], in_=isel[:], op=OP.min,
                                axis=mybir.AxisListType.X)
        rm = wk.tile([16, 64], U32, tag="rm")
        nc.vector.tensor_tensor(out=rm[:], in0=CI,
                                in1=c1[:].to_broadcast([16, 64]), op=OP.is_equal)
        CV2 = wk.tile([16, 64], F32, tag="CV2")
        nc.vector.select(CV2[:], rm[:], negt[:], CVf)
        m8b = wk.tile([16, 8], F32, tag="m8b")
        nc.vector.max(out=m8b[:], in_=CV2[:])
        eq2 = wk.tile([16, 64], U32, tag="eq2")
        nc.vector.tensor_tensor(out=eq2[:], in0=CV2[:],
                                in1=m8b[:, 0:1].to_broadcast([16, 64]), op=OP.is_equal)
        isel2 = wk.tile([16, 64], F32, tag="isel2")
        nc.vector.select(isel2[:], eq2[:], CI, bigt[:])
        c2 = wk.tile([16, 1], F32, tag="c2")
        nc.vector.tensor_reduce(out=c2[:], in_=isel2[:], op=OP.min,
                                axis=mybir.AxisListType.X)

        cidx = wk.tile([16, 2], F32, tag="cidx")
        nc.vector.tensor_copy(cidx[:, 0:1], c1[:])
        nc.vector.tensor_copy(cidx[:, 1:2], c2[:])
        nc.vector.tensor_copy(prob[:, 0:1], m8[:, 0:1])
        nc.vector.tensor_copy(prob[:, 1:2], m8b[:, 0:1])
        parent = wk.tile([16, 2], F32, tag="parent")
        nc.vector.tensor_scalar(parent[:], cidx[:], 32000.0, scalar2=None,
                                op0=OP.is_ge)
        token = wk.tile([16, 2], F32, tag="token")
        nc.vector.scalar_tensor_tensor(out=token[:], in0=parent[:], scalar=-32000.0,
                                       in1=cidx[:], op0=OP.mult, op1=OP.add)

        pmask = wk.tile([16, 2], U32, tag="pmask")
        nc.vector.tensor_copy(pmask[:], parent[:])
        for hist, val in ((th, token), (sh, prob)):
            hnew = wk.tile([16, 2, T], F32, tag="hnew")
            pb = apb(pmask[:], [pmask[:].ap[0], pmask[:].ap[1], [0, T]])
            h0ap = hist[:, 0, :]
            h1ap = hist[:, 1, :]
            on0 = apb(h0ap, [h0ap.ap[0], [0, 2], h0ap.ap[1]])
            on1 = apb(h1ap, [h1ap.ap[0], [0, 2], h1ap.ap[1]])
            nc.vector.tensor_copy(hnew[:], on0)
            nc.vector.copy_predicated(hnew[:], pb, on1)
            nc.vector.tensor_copy(hist[:], hnew[:])
            vap = apb(val[:], [val[:].ap[0], val[:].ap[1], [1, 1]])
            nc.vector.tensor_copy(hist[:, :, t:t + 1], vap)

        p32p = ps.tile([128, 512], F32, tag="pscr")
        nc.tensor.matmul(p32p[0:32, 0:1], sk0[:], parent[:, 0:1], start=True, stop=False)
        nc.tensor.matmul(p32p[0:32, 0:1], sk1[:], parent[:, 1:2], start=False, stop=True)
        srcrow = wk.tile([32, 1], F32, tag="srcrow")
        nc.vector.tensor_add(srcrow[:], p32p[0:32, 0:1], rowbase[:])
        srTp = ps.tile([128, 512], F32, tag="pscr")
        nc.tensor.transpose(srTp[0:1, 0:32], srcrow[:], ident[0:32, 0:32])
        srTs = wk.tile([1, 32], F32, tag="srTs")
        nc.vector.tensor_copy(srTs[:], srTp[0:1, 0:32])
        srRp = ps.tile([128, 512], F32, tag="pscr")
        nc.tensor.matmul(srRp[0:32, 0:32], onesr[:], srTs[:], start=True, stop=True)
        PT = wk.tile([32, 32], F32, tag="PT")
        nc.vector.tensor_tensor(out=PT[:], in0=rowid[:].to_broadcast([32, 32]),
                                in1=srRp[0:32, 0:32], op=OP.is_equal)
        hselp = ps.tile([128, 512], F32, tag="pscr")
        nc.tensor.matmul(hselp[0:32, :], PT[:], newh[:], start=True, stop=True)
        nc.vector.tensor_copy(hid[:], hselp[0:32, :])
        for n in range(3):
            gselp = ps.tile([128, 512], F32, tag="pscr")
            nc.tensor.matmul(gselp[0:32, :], PT[:], ghb[:, n * 512:(n + 1) * 512],
                             start=True, stop=True)
            nc.vector.tensor_copy(ghb[:, n * 512:(n + 1) * 512], gselp[0:32, :])

        t32p = ps.tile([128, 512], F32, tag="pscr")
        nc.tensor.matmul(t32p[0:32, 0:1], sk0[:], token[:, 0:1], start=True, stop=False)
        nc.tensor.matmul(t32p[0:32, 0:1], sk1[:], token[:, 1:2], start=False, stop=True)
        nc.vector.tensor_copy(tok_i[:], t32p[0:32, 0:1])

    best = wk.tile([16, 1], U32, tag="best")
    d01 = wk.tile([16, 1], F32, tag="d01")
    nc.vector.tensor_sub(d01[:], prob[:, 1:2], prob[:, 0:1])
    nc.vector.tensor_scalar(best[:], d01[:], 0.0, scalar2=None, op0=OP.is_gt)
    tk = wk.tile([16, T], F32, tag="tk")
    sc = wk.tile([16, T], F32, tag="sc")
    bb = apb(best[:], [best[:].ap[0], [0, T]])
    nc.vector.tensor_copy(tk[:], th[:, 0, :])
    nc.vector.copy_predicated(tk[:], bb, th[:, 1, :])
    nc.vector.tensor_copy(sc[:], sh[:, 0, :])
    nc.vector.copy_predicated(sc[:], bb, sh[:, 1, :])
    tki = wk.tile([16, T], I32, tag="tki")
    nc.vector.tensor_copy(tki[:], tk[:])
    nc.sync.dma_start(out_tok.ap(), tki[:])
    nc.sync.dma_start(out_sc.ap(), sc[:])
    ctx.close()


# ===================== host side =====================

_CACHED = {}
LAST = {}


def kernel(**inputs):
    f32 = np.float32
    src = np.asarray(inputs["src"]).astype(np.int64)
    W_out = np.asarray(inputs["W_out"], f32)
    b_out = np.asarray(inputs["b_out"], f32)
    enc_bih = np.asarray(inputs["enc_bih"], f32)
    enc_bhh = np.asarray(inputs["enc_bhh"], f32)
    dec_bih = np.asarray(inputs["dec_bih"], f32)
    dec_bhh = np.asarray(inputs["dec_bhh"], f32)

    if "nc" not in _CACHED:
        _CACHED["nc"] = build_kernel()
    nc = _CACHED["nc"]

    srcw = np.zeros((128, 16), np.int32)
    flat = src.reshape(L * B)
    for j in range(16):
        srcw[:, j] = flat[j * 128:(j + 1) * 128]
    sosidx = np.full((32, 1), SOS, np.int32)
    benc = enc_bih + enc_bhh
    bdec = dec_bih + dec_bhh
    ident = np.eye(128, dtype=f32)
    sk0 = np.zeros((16, 32), f32)
    sk1 = np.zeros((16, 32), f32)
    for b in range(16):
        sk0[b, 2 * b] = 1.0
        sk1[b, 2 * b + 1] = 1.0
    common = {
        "srcw": srcw, "sosidx": sosidx,
        "embenc": np.asarray(inputs["emb_enc"], f32),
        "embdec": np.asarray(inputs["emb_dec"], f32),
        "wihte": np.ascontiguousarray(np.asarray(inputs["enc_Wih"], f32).T),
        "whhte": np.ascontiguousarray(np.asarray(inputs["enc_Whh"], f32).T),
        "krz_enc": np.broadcast_to(benc[:1024], (16, 1024)).copy(),
        "kin_enc": np.broadcast_to(enc_bih[1024:], (16, 512)).copy(),
        "khn_enc": np.broadcast_to(enc_bhh[1024:], (16, 512)).copy(),
        "wihtde": np.ascontiguousarray(np.asarray(inputs["dec_Wih"], f32)[:, :E].T),
        "wihtdc": np.ascontiguousarray(np.asarray(inputs["dec_Wih"], f32)[:, E:].T),
        "whhtd": np.ascontiguousarray(np.asarray(inputs["dec_Whh"], f32).T),
        "krz_dec": np.broadcast_to(bdec[:1024], (32, 1024)).copy(),
        "kin_dec": np.broadcast_to(dec_bih[1024:], (32, 512)).copy(),
        "khn_dec": np.broadcast_to(dec_bhh[1024:], (32, 512)).copy(),
        "ident": ident, "sk0": sk0, "sk1": sk1,
        "skT0": np.ascontiguousarray(sk0.T), "skT1": np.ascontiguousarray(sk1.T),
        "rowbase": (2 * (np.arange(32) // 2)).astype(f32).reshape(32, 1),
        "rowid": np.arange(32, dtype=f32).reshape(32, 1),
        "bigt": np.full((16, 64), BIGC, f32),
        "negt": np.full((16, 64), NEGC, f32),
    }
    in_maps = []
    for core in range(NC):
        Wsh = np.zeros((VSP, E + 2 * H), f32)
        Wsh[:VS] = W_out[core * VS:(core + 1) * VS]
        bsh = np.full((1, VSP), -30000.0, f32)
        bsh[0, :VS] = b_out[core * VS:(core + 1) * VS]
        kofs = ((np.arange(32) % 2) * 32000.0 + core * 4000.0).astype(f32).reshape(32, 1)
        sadd = np.zeros((16, 2, NC, 4), f32)
        sadd[:, :, 1:, 2:4] = NEGC
        im = dict(common)
        im.update({
            "wote": np.ascontiguousarray(Wsh[:, :768].T),
            "wotc": np.ascontiguousarray(Wsh[:, 768:].T),
            "bout": bsh, "kofs": kofs, "sadd": sadd.reshape(16, 64),
        })
        in_maps.append(im)

    import os
    trace = bool(os.environ.get("BASS_TRACE"))
    res = run_bass_kernel_spmd(nc, in_maps, core_ids=list(range(NC)), trace=trace)
    LAST["exec_time_ns"] = res.exec_time_ns
    r0 = res.results[0]
    return np.asarray(r0["out_tok"], np.int32), np.asarray(r0["out_sc"], np.float32)


if __name__ == "__main__":
    import reference as R
    inp = {k: np.asarray(v) for k, v in R.setup_inputs().items()}
    tok, sc = kernel(**inp)
    exp_t = np.load("exp_tok.npy")
    exp_s = np.load("exp_sc.npy")
    nm = int((tok != exp_t).sum())
    print("token mismatches:", nm, "/", tok.size)
    rel = np.abs(sc - exp_s) / np.maximum(np.abs(exp_s), 1e-38)
    print("score max rel err (|exp|>1e-30):", float(np.nanmax(np.where(np.abs(exp_s) > 1e-30, rel, 0))))
    print(tok[0])
